# revision 40
# baseline (speedup 1.0000x reference)
"""Trainium2 Bass kernel for BaseCausalWanSelfAttention (local+sink sparse attention
with interleaved rotary), SPMD across 8 NeuronCores.

Sharding: the 24 (batch, head) pairs are split 3-per-core across 8 cores; each
core runs full local+sink attention for its pairs independently (no collectives).

k-major fp16 design: scores are [key-partition, query-free] so PV and the
softmax-denominator both run as matmuls; all SBUF data is fp16 (1 cyc/row on
the PE like bf16, 2-4x DVE modes); rotary uses a host-prepared pair-swapped
copy of q/k so it is 3 pure-DVE ops with no PE/PSUM involvement.
"""
import sys

sys.path.insert(0, "/opt/trn_rl_repo")

import numpy as np

import concourse.bacc as bacc
import concourse.bass_isa as bass_isa
import concourse.tile as tile
import concourse.mybir as mybir

dt = mybir.dt

# Problem config (hardcoded per contest contract)
B, S, H, D = 2, 3072, 12, 128
LOCAL_WINDOW = 1560
SINK = 128
N_CORES = 8
PER_CORE = (B * H) // N_CORES  # 3
QB = 512  # query block (columns of transposed scores)
NQC = QB // 128  # 128-query chunks per block
SCALE = 1.0 / float(np.sqrt(D))
# fp16 Schraudolph exp on DVE: exp(s*SCALE) ~ bitcast_f16(int16(s*SCHRA_A + SCHRA_B))
SCHRA_A = (1024.0 / float(np.log(2))) * SCALE
SCHRA_B = 15360.0 - 45.0
DVE_EXP_EVERY = 4  # every Nth score tile takes the DVE Schraudolph exp path


def _window_partial_deltas(w):
    """k-tile offsets (qi - kj) where the local-window edge cuts through the
    128x128 tile; maps delta -> threshold T with allowed iff (c - p) < T."""
    out = {}
    for d in range((w - 127 + 127) // 128, (w + 127) // 128 + 1):
        t = w - 128 * d
        if -127 <= t <= 127:
            out[d] = t
    return out


def chunk_kinds(qb, kj, w=LOCAL_WINDOW, nqc=NQC):
    """Per 128-query chunk classification of k-tile kj for query block qb.
    Returns list of (t, kind) with kind in {"full", "diag", ("win", delta)} for
    valid chunks only. SINK==128 assumed (k-tile 0 fully attendable)."""
    partial = _window_partial_deltas(w)
    max_delta = max(partial) if partial else (w - 1) // 128
    kinds = []
    for t in range(nqc):
        qi = nqc * qb + t
        if kj == 0:
            kinds.append((t, "diag" if qi == 0 else "full"))
            continue
        delta = qi - kj
        if delta < 0 or delta > max_delta:
            continue
        if delta == 0:
            kinds.append((t, "diag"))
        elif delta in partial:
            kinds.append((t, ("win", delta)))
        else:
            kinds.append((t, "full"))
    return kinds


def kj_list(qb, s=S, w=LOCAL_WINDOW, nqc=NQC):
    partial = _window_partial_deltas(w)
    max_delta = max(partial) if partial else (w - 1) // 128
    n_ktiles = s // 128
    hi = min(nqc * qb + nqc - 1, n_ktiles - 1)
    lo = max(1, nqc * qb - max_delta)
    return [0] + [kj for kj in range(lo, hi + 1)]




def mask_patterns(s=S, w=LOCAL_WINDOW):
    """Distinct per-tile chunk-kind patterns that contain any masked chunk.
    Returns {kinds_key: [128, 512] fp16 additive-bias array} where valid slots
    hold 15312 and masked slots 15312 - 15360 (bitcasts to ~0 in fp16)."""
    import numpy as np
    partial = _window_partial_deltas(w)
    nqb = s // QB
    pats = {}
    p = np.arange(128)[:, None]
    c = np.arange(128)[None, :]
    for qb in range(nqb):
        for kj in kj_list(qb, s=s, w=w):
            kinds = chunk_kinds(qb, kj, w=w)
            key = tuple(k for _, k in kinds)
            if all(k == "full" for k in key) or key in pats:
                continue
            arr = np.full((128, 512), 15312.0, dtype=np.float64)
            for i, (_, kind) in enumerate(kinds):
                if kind == "full":
                    continue
                valid = (c >= p) if kind == "diag" else ((c - p) < partial[kind[1]])
                blk = np.where(valid, 15312.0, 15312.0 - 15360.0)
                arr[:, i * 128 : (i + 1) * 128] = blk
            pats[key] = arr.astype(np.float16)
    return pats

def build_nc(s=S, per_core=PER_CORE, w=LOCAL_WINDOW):
    """Build the SPMD single-core program (identical on all cores)."""
    nqb = s // QB
    partial = _window_partial_deltas(w)
    f16 = dt.float16

    nc = bacc.Bacc("TRN2", target_bir_lowering=False, debug=False)

    qT = nc.declare_dram_parameter("qT", [per_core, 128, s], f16, isOutput=False)
    qTs = nc.declare_dram_parameter("qTs", [per_core, 128, s], f16, isOutput=False)
    kT = nc.declare_dram_parameter("kT", [per_core, 128, s], f16, isOutput=False)
    kTs = nc.declare_dram_parameter("kTs", [per_core, 128, s], f16, isOutput=False)
    v = nc.declare_dram_parameter("v", [per_core, s, 128], f16, isOutput=False)
    cexpT = nc.declare_dram_parameter("cexpT", [128, s], f16, isOutput=False)
    ssigT = nc.declare_dram_parameter("ssigT", [128, s], f16, isOutput=False)
    ident = nc.declare_dram_parameter("ident", [128, 128], f16, isOutput=False)
    ones = nc.declare_dram_parameter("ones", [128, 128], f16, isOutput=False)
    maskD = nc.declare_dram_parameter("maskD", [128, 128], f16, isOutput=False)
    wmask_names = {}
    for delta in sorted(partial):
        wmask_names[delta] = nc.declare_dram_parameter(f"maskW{delta}", [128, 128], f16, isOutput=False)
    out = nc.declare_dram_parameter("out", [per_core, s, 128], f16, isOutput=True)

    with tile.TileContext(nc) as tc:
        with (
            tc.tile_pool(name="const", bufs=1) as cpool,
            tc.tile_pool(name="big", bufs=2) as bigpool,
            tc.tile_pool(name="probs", bufs=12) as ppool,
            tc.tile_pool(name="tail", bufs=3) as tpool,
            tc.tile_pool(name="ps_sc", bufs=5, space="PSUM") as ps_sc,
            tc.tile_pool(name="ps_out", bufs=2, space="PSUM") as ps_out,
            tc.tile_pool(name="ps_den", bufs=1, space="PSUM") as ps_den,
        ):
            # constants
            cexp_sb = cpool.tile([128, s], f16, tag="cexp")
            ssig_sb = cpool.tile([128, s], f16, tag="ssig")
            nc.sync.dma_start(out=cexp_sb[:, 0:1024], in_=cexpT[:, 0:1024])
            nc.sync.dma_start(out=ssig_sb[:, 0:1024], in_=ssigT[:, 0:1024])
            ident_sb = cpool.tile([128, 128], f16, tag="ident")
            ones_sb = cpool.tile([128, 128], f16, tag="ones")
            maskD_sb = cpool.tile([128, 128], f16, tag="maskD")
            wdeltas = sorted(wmask_names)
            wmask_sb = {
                delta: cpool.tile([128, 128], f16, tag=f"maskW{delta}", name=f"mW{delta}")
                for delta in wdeltas
            }
            wpair_sb = None
            if len(wdeltas) == 2 and wdeltas[1] == wdeltas[0] + 1:
                wpair_sb = cpool.tile([128, 256], f16, tag="maskWpair")

            def load_consts_rest():
                nc.sync.dma_start(out=ident_sb[:], in_=ident[:])
                nc.sync.dma_start(out=ones_sb[:], in_=ones[:])
                nc.sync.dma_start(out=maskD_sb[:], in_=maskD[:])
                for delta, m in wmask_sb.items():
                    nc.sync.dma_start(out=m[:], in_=wmask_names[delta][:])
                if wpair_sb is not None:
                    nc.sync.dma_start(out=wpair_sb[:, 0:128], in_=wmask_names[wdeltas[0]][:])
                    nc.sync.dma_start(out=wpair_sb[:, 128:256], in_=wmask_names[wdeltas[1]][:])
                for c2 in range(1, s // 1024):
                    sl2 = slice(c2 * 1024, (c2 + 1) * 1024)
                    nc.sync.dma_start(out=cexp_sb[:, sl2], in_=cexpT[:, sl2])
                    nc.sync.dma_start(out=ssig_sb[:, sl2], in_=ssigT[:, sl2])

            def load(u, first=False):
                qraw = bigpool.tile([128, s], f16, tag="qraw")
                qsw = bigpool.tile([128, s], f16, tag="qsw")
                kraw = bigpool.tile([128, s], f16, tag="kraw")
                ksw = bigpool.tile([128, s], f16, tag="ksw")
                v_sb = bigpool.tile([128, s], f16, tag="v")
                for c2 in range(s // 1024):
                    sl2 = slice(c2 * 1024, (c2 + 1) * 1024)
                    nc.sync.dma_start(out=qraw[:, sl2], in_=qT[u][:, sl2])
                    nc.sync.dma_start(out=qsw[:, sl2], in_=qTs[u][:, sl2])
                    nc.sync.dma_start(out=kraw[:, sl2], in_=kT[u][:, sl2])
                    nc.sync.dma_start(out=ksw[:, sl2], in_=kTs[u][:, sl2])
                nc.sync.dma_start(
                    out=v_sb[:].rearrange("p (n d) -> p n d", d=128),
                    in_=v[u].rearrange("(n p) d -> p n d", p=128),
                )
                rq = bigpool.tile([128, s], f16, tag="rq")
                rk = bigpool.tile([128, s], f16, tag="rk")
                return qraw, qsw, kraw, ksw, v_sb, rq, rk

            def rotary(tiles, lo, hi):
                """Rotary for columns [lo,hi): r = raw*cexp + swapped*ssig.
                raw is reused as scratch for the second product."""
                qraw, qsw, kraw, ksw, v_sb, rq, rk = tiles
                step = 1024 if (hi - lo) % 1024 == 0 else 512
                for raw, sw, r in ((qraw, qsw, rq), (kraw, ksw, rk)):
                    for c in range(lo // step, hi // step):
                        sl = slice(c * step, (c + 1) * step)
                        nc.vector.tensor_mul(r[:, sl], raw[:, sl], cexp_sb[:, sl])
                        nc.vector.tensor_mul(raw[:, sl], sw[:, sl], ssig_sb[:, sl])
                        nc.vector.tensor_add(r[:, sl], r[:, sl], raw[:, sl])

            def emit_masks(probs, kinds, off):
                mk = [k for k in kinds if k[1] != "full"]
                j = 0
                while j < len(mk):
                    t, kind = mk[j]
                    if (
                        wpair_sb is not None
                        and j + 1 < len(mk)
                        and kind != "diag"
                        and mk[j + 1][1] != "diag"
                        and mk[j + 1][0] == t + 1
                        and kind[1] == wdeltas[0]
                    ):
                        tsl = slice(off + t * 128, off + (t + 2) * 128)
                        nc.vector.tensor_mul(probs[:, tsl], probs[:, tsl], wpair_sb[:])
                        j += 2
                        continue
                    m = maskD_sb if kind == "diag" else wmask_sb[kind[1]]
                    tsl = slice(off + t * 128, off + (t + 1) * 128)
                    nc.vector.tensor_mul(probs[:, tsl], probs[:, tsl], m[:])
                    j += 1

            def qb_order(qb):
                kjs = kj_list(qb, s=s, w=w)
                tiles = []
                for kj in kjs:
                    kinds = chunk_kinds(qb, kj, w=w)
                    assert kinds, (qb, kj)
                    tiles.append((kj, kinds, kinds[0][0], kinds[-1][0] + 1))
                fulls = [x for x in tiles if x[3] - x[2] == NQC]
                parts = [x for x in tiles if x[3] - x[2] != NQC]
                assert fulls[0][0] == 0
                order = [fulls[0]]
                rest_f = fulls[1:]
                rest_p = list(parts)
                stride = (
                    max(1, len(rest_f) // (len(rest_p) + 1)) if rest_p else len(rest_f) or 1
                )
                while rest_f or rest_p:
                    order.extend(rest_f[:stride])
                    rest_f = rest_f[stride:]
                    if rest_p:
                        order.append(rest_p.pop(0))
                return order

            WAVE = 4
            state = {"pv": [], "tail": None, "exp_ctr": 0}

            def flush_pv():
                if state["pv"]:
                    state["pv"].pop(0)()

            def flush_all():
                while state["pv"]:
                    flush_pv()

            def attention_qb(u, rq, rk, v_sb, qb):
                order = qb_order(qb)
                n_tiles = len(order)
                qbctx = {}

                def get_psums():
                    if "outT" not in qbctx:
                        outT_ps = ps_out.tile([128, QB], dt.float32, tag="outT")
                        den_ps = ps_den.tile([128, QB], dt.float32, tag="den")
                        qbctx["outT"] = outT_ps
                        qbctx["den"] = den_ps
                    return qbctx["outT"], qbctx["den"]

                for w0 in range(0, n_tiles, WAVE):
                    wave = order[w0 : w0 + WAVE]
                    wprobs = []
                    for kj, kinds, t0, t1 in wave:
                        csl = slice(qb * QB + t0 * 128, qb * QB + t1 * 128)
                        psl = slice(t0 * 128, t1 * 128)
                        ksl = slice(kj * 128, (kj + 1) * 128)
                        sc = ps_sc.tile([128, QB], dt.float32, tag="sc")
                        nc.tensor.matmul(
                            sc[:, psl], rk[:, ksl], rq[:, csl], start=True, stop=True
                        )
                        probs = ppool.tile([128, QB], f16, tag="probs")
                        state["exp_ctr"] += 1
                        if state["exp_ctr"] % DVE_EXP_EVERY == 0:
                            nc.vector.tensor_scalar(
                                probs[:, psl].bitcast(dt.int16),
                                sc[:, psl],
                                SCHRA_A,
                                SCHRA_B,
                                op0=mybir.AluOpType.mult,
                                op1=mybir.AluOpType.add,
                            )
                        else:
                            nc.scalar.activation(
                                probs[:, psl],
                                sc[:, psl],
                                mybir.ActivationFunctionType.Exp,
                                scale=SCALE,
                            )
                        emit_masks(probs, kinds, 0)
                        wprobs.append(probs)

                    is_last_wave = w0 + WAVE >= n_tiles

                    def pv_emit(
                        u=u, qb=qb, wave=wave, wprobs=wprobs,
                        w0=w0, n_tiles=n_tiles, last_wave=is_last_wave,
                    ):
                        outT_ps, den_ps = get_psums()
                        for wi, (kj, kinds, t0, t1) in enumerate(wave):
                            psl = slice(t0 * 128, t1 * 128)
                            ksl = slice(kj * 128, (kj + 1) * 128)
                            first = kj == 0
                            last = w0 + wi == n_tiles - 1
                            nc.tensor.matmul(
                                outT_ps[:, psl], v_sb[:, ksl], wprobs[wi][:, psl],
                                start=first, stop=last,
                            )
                            nc.tensor.matmul(
                                den_ps[:, psl], ones_sb[:], wprobs[wi][:, psl],
                                start=first, stop=last,
                            )
                        if last_wave:
                            # normalize now; transposes/store deferred one qb
                            rden = tpool.tile([128, QB], dt.float32, tag="rden")
                            nc.vector.reciprocal_approx_fast(rden[:], den_ps[:])
                            outN = tpool.tile([128, QB], f16, tag="outN")
                            nc.vector.tensor_mul(outN[:], outT_ps[:], rden[:])

                            def tail(u=u, qb=qb, outN=outN, chunked=False):
                                tr = ps_sc.tile([128, QB], f16, tag="sc")
                                out_sb = tpool.tile([128, QB], f16, tag="out_sb")
                                out_v = out[u].rearrange("(n p) d -> p n d", p=128)
                                if chunked:
                                    for c in range(NQC):
                                        tsl = slice(c * 128, (c + 1) * 128)
                                        nc.tensor.transpose(
                                            tr[:, tsl], outN[:, tsl], ident_sb[:]
                                        )
                                        nc.vector.tensor_copy(
                                            out_sb[:, tsl], tr[:, tsl]
                                        )
                                        nc.sync.dma_start(
                                            out=out_v[:, qb * NQC + c, :],
                                            in_=out_sb[:, tsl],
                                        )
                                    return
                                for c in range(NQC):
                                    tsl = slice(c * 128, (c + 1) * 128)
                                    nc.tensor.transpose(
                                        tr[:, tsl], outN[:, tsl], ident_sb[:]
                                    )
                                nc.vector.tensor_copy(out_sb[:], tr[:])
                                nc.sync.dma_start(
                                    out=out_v[:, qb * NQC : (qb + 1) * NQC, :],
                                    in_=out_sb[:].rearrange("p (n d) -> p n d", d=128),
                                )

                            if state["tail"] is not None:
                                state["tail"]()
                            state["tail"] = tail

                    state["pv"].append(pv_emit)
                    flush_pv() if len(state["pv"]) > 1 else None

            cur = load(0)
            load_consts_rest()
            # warm the ACT exp table during the load phase so the first real
            # exp doesn't pay the table-load latency
            warm = tpool.tile([128, 1], f16, tag="warm")
            nc.scalar.activation(
                warm[:], ones_sb[:, 0:1], mybir.ActivationFunctionType.Exp, scale=1.0
            )
            for u in range(per_core):
                nxt = load(u + 1) if u + 1 < per_core else None
                if u == 0:
                    rotary(cur, 0, QB)
                for qb in range(nqb):
                    if u == 0 and qb + 1 < nqb:
                        rotary(cur, (qb + 1) * QB, (qb + 2) * QB)
                    if nxt is not None:
                        # spread next pair's rotary across this pair's qbs so no
                        # DVE lump blocks the pair boundary
                        rotary(nxt, qb * QB, (qb + 1) * QB)
                    attention_qb(u, cur[5], cur[6], cur[4], qb)
                cur = nxt
            flush_all()
            if state["tail"] is not None:
                state["tail"](chunked=True)

    nc.compile()
    return nc


def host_prep(q, k, v, cos, sin, s=S, w=LOCAL_WINDOW):
    """Build per-core input maps from full inputs (fp16 device layouts)."""
    b, _, h, d = q.shape
    partial = _window_partial_deltas(w)
    f16 = np.float16

    cexp = np.empty((128, s), dtype=np.float32)
    ssig = np.empty((128, s), dtype=np.float32)
    cexp[0::2, :] = cos.T
    cexp[1::2, :] = cos.T
    ssig[0::2, :] = -sin.T
    ssig[1::2, :] = sin.T

    ident = np.eye(128, dtype=np.float32)
    ones = np.ones((128, 128), dtype=np.float32)

    p = np.arange(128)[:, None]
    c = np.arange(128)[None, :]
    maskD = (c >= p).astype(np.float32)
    wmasks = {delta: ((c - p) < t).astype(np.float32) for delta, t in partial.items()}

    perm = np.arange(128) ^ 1  # rotary pair swap on the d axis

    units = [(bi, hi) for bi in range(b) for hi in range(h)]
    per = len(units) // N_CORES
    in_maps = []
    for core in range(N_CORES):
        us = units[core * per : (core + 1) * per]
        qTc = np.stack([q[bi, :, hi, :].T for bi, hi in us]).astype(f16)
        kTc = np.stack([k[bi, :, hi, :].T for bi, hi in us]).astype(f16)
        vc = np.stack([v[bi, :, hi, :] for bi, hi in us]).astype(f16)
        m = {
            "qT": np.ascontiguousarray(qTc),
            "qTs": np.ascontiguousarray(qTc[:, perm, :]),
            "kT": np.ascontiguousarray(kTc),
            "kTs": np.ascontiguousarray(kTc[:, perm, :]),
            "v": np.ascontiguousarray(vc),
            "cexpT": cexp.astype(f16),
            "ssigT": ssig.astype(f16),
            "ident": ident.astype(f16),
            "ones": ones.astype(f16),
            "maskD": maskD.astype(f16),
        }
        for delta, msk in wmasks.items():
            m[f"maskW{delta}"] = msk.astype(f16)
        in_maps.append(m)
    return in_maps, units


_NC_CACHE = {}


def kernel(q, k, v, cos, sin):
    from concourse.bass_utils import run_bass_kernel_spmd

    q = np.asarray(q, dtype=np.float32)
    k = np.asarray(k, dtype=np.float32)
    v = np.asarray(v, dtype=np.float32)
    cos = np.asarray(cos, dtype=np.float32)
    sin = np.asarray(sin, dtype=np.float32)

    if "nc" not in _NC_CACHE:
        _NC_CACHE["nc"] = build_nc()
    nc = _NC_CACHE["nc"]

    in_maps, units = host_prep(q, k, v, cos, sin)
    res = run_bass_kernel_spmd(nc, in_maps, core_ids=list(range(N_CORES)))

    b, s, h, d = q.shape
    full = np.empty((b, s, h, d), dtype=np.float32)
    per = len(units) // N_CORES
    for core in range(N_CORES):
        o = np.asarray(res.results[core]["out"], dtype=np.float32)  # [per, s, 128]
        for i, (bi, hi) in enumerate(units[core * per : (core + 1) * per]):
            full[bi, :, hi, :] = o[i]
    return full


# revision 41
# speedup vs baseline: 1.2564x; 1.2564x over previous
"""Trainium2 Bass kernel for BaseCausalWanSelfAttention (local+sink sparse attention
with interleaved rotary), SPMD across 8 NeuronCores.

Sharding: the 24 (batch, head) pairs are split 3-per-core across 8 cores; each
core runs full local+sink attention for its pairs independently (no collectives).

k-major fp16 design: scores are [key-partition, query-free] so PV and the
softmax-denominator both run as matmuls; all SBUF data is fp16 (1 cyc/row on
the PE like bf16, 2-4x DVE modes); rotary uses a host-prepared pair-swapped
copy of q/k so it is 3 pure-DVE ops with no PE/PSUM involvement.
"""
import sys

sys.path.insert(0, "/opt/trn_rl_repo")

import numpy as np

import concourse.bacc as bacc
import concourse.bass_isa as bass_isa
import concourse.tile as tile
import concourse.mybir as mybir

dt = mybir.dt

# Problem config (hardcoded per contest contract)
B, S, H, D = 2, 3072, 12, 128
LOCAL_WINDOW = 1560
SINK = 128
N_CORES = 8
PER_CORE = (B * H) // N_CORES  # 3
QB = 512  # query block (columns of transposed scores)
NQC = QB // 128  # 128-query chunks per block
SCALE = 1.0 / float(np.sqrt(D))
# fp16 Schraudolph exp on DVE: exp(s*SCALE) ~ bitcast_f16(int16(s*SCHRA_A + SCHRA_B))
SCHRA_A = (1024.0 / float(np.log(2))) * SCALE
SCHRA_B = 15360.0 - 45.0
DVE_EXP_EVERY = 4  # every Nth score tile takes the DVE Schraudolph exp path


def _window_partial_deltas(w):
    """k-tile offsets (qi - kj) where the local-window edge cuts through the
    128x128 tile; maps delta -> threshold T with allowed iff (c - p) < T."""
    out = {}
    for d in range((w - 127 + 127) // 128, (w + 127) // 128 + 1):
        t = w - 128 * d
        if -127 <= t <= 127:
            out[d] = t
    return out


def chunk_kinds(qb, kj, w=LOCAL_WINDOW, nqc=NQC):
    """Per 128-query chunk classification of k-tile kj for query block qb.
    Returns list of (t, kind) with kind in {"full", "diag", ("win", delta)} for
    valid chunks only. SINK==128 assumed (k-tile 0 fully attendable)."""
    partial = _window_partial_deltas(w)
    max_delta = max(partial) if partial else (w - 1) // 128
    kinds = []
    for t in range(nqc):
        qi = nqc * qb + t
        if kj == 0:
            kinds.append((t, "diag" if qi == 0 else "full"))
            continue
        delta = qi - kj
        if delta < 0 or delta > max_delta:
            continue
        if delta == 0:
            kinds.append((t, "diag"))
        elif delta in partial:
            kinds.append((t, ("win", delta)))
        else:
            kinds.append((t, "full"))
    return kinds


def kj_list(qb, s=S, w=LOCAL_WINDOW, nqc=NQC):
    partial = _window_partial_deltas(w)
    max_delta = max(partial) if partial else (w - 1) // 128
    n_ktiles = s // 128
    hi = min(nqc * qb + nqc - 1, n_ktiles - 1)
    lo = max(1, nqc * qb - max_delta)
    return [0] + [kj for kj in range(lo, hi + 1)]




def mask_patterns(s=S, w=LOCAL_WINDOW):
    """Distinct per-tile chunk-kind patterns that contain any masked chunk.
    Returns {kinds_key: [128, 512] fp16 additive-bias array} where valid slots
    hold 15312 and masked slots 15312 - 15360 (bitcasts to ~0 in fp16)."""
    import numpy as np
    partial = _window_partial_deltas(w)
    nqb = s // QB
    pats = {}
    p = np.arange(128)[:, None]
    c = np.arange(128)[None, :]
    for qb in range(nqb):
        for kj in kj_list(qb, s=s, w=w):
            kinds = chunk_kinds(qb, kj, w=w)
            key = tuple(k for _, k in kinds)
            if all(k == "full" for k in key) or key in pats:
                continue
            arr = np.full((128, 512), 15312.0, dtype=np.float64)
            for i, (_, kind) in enumerate(kinds):
                if kind == "full":
                    continue
                valid = (c >= p) if kind == "diag" else ((c - p) < partial[kind[1]])
                blk = np.where(valid, 15312.0, 15312.0 - 15360.0)
                arr[:, i * 128 : (i + 1) * 128] = blk
            pats[key] = arr.astype(np.float16)
    return pats

def build_nc(s=S, per_core=PER_CORE, w=LOCAL_WINDOW):
    """Build the SPMD single-core program (identical on all cores)."""
    nqb = s // QB
    partial = _window_partial_deltas(w)
    f16 = dt.float16

    nc = bacc.Bacc("TRN2", target_bir_lowering=False, debug=False)

    qT = nc.declare_dram_parameter("qT", [per_core, 128, s], f16, isOutput=False)
    qTs = nc.declare_dram_parameter("qTs", [per_core, 128, s], f16, isOutput=False)
    kT = nc.declare_dram_parameter("kT", [per_core, 128, s], f16, isOutput=False)
    kTs = nc.declare_dram_parameter("kTs", [per_core, 128, s], f16, isOutput=False)
    v = nc.declare_dram_parameter("v", [per_core, s, 128], f16, isOutput=False)
    cexpT = nc.declare_dram_parameter("cexpT", [128, s], f16, isOutput=False)
    ssigT = nc.declare_dram_parameter("ssigT", [128, s], f16, isOutput=False)
    ident = nc.declare_dram_parameter("ident", [128, 128], f16, isOutput=False)
    ones = nc.declare_dram_parameter("ones", [128, 128], f16, isOutput=False)
    maskD = nc.declare_dram_parameter("maskD", [128, 128], f16, isOutput=False)
    wmask_names = {}
    for delta in sorted(partial):
        wmask_names[delta] = nc.declare_dram_parameter(f"maskW{delta}", [128, 128], f16, isOutput=False)
    out = nc.declare_dram_parameter("out", [per_core, s, 128], f16, isOutput=True)

    with tile.TileContext(nc) as tc:
        with (
            tc.tile_pool(name="const", bufs=1) as cpool,
            tc.tile_pool(name="big", bufs=2) as bigpool,
            tc.tile_pool(name="probs", bufs=12) as ppool,
            tc.tile_pool(name="tail", bufs=3) as tpool,
            tc.tile_pool(name="ps_sc", bufs=5, space="PSUM") as ps_sc,
            tc.tile_pool(name="ps_out", bufs=2, space="PSUM") as ps_out,
            tc.tile_pool(name="ps_den", bufs=1, space="PSUM") as ps_den,
        ):
            # constants
            cexp_sb = cpool.tile([128, s], f16, tag="cexp")
            ssig_sb = cpool.tile([128, s], f16, tag="ssig")
            nc.sync.dma_start(out=cexp_sb[:, 0:1024], in_=cexpT[:, 0:1024])
            nc.sync.dma_start(out=ssig_sb[:, 0:1024], in_=ssigT[:, 0:1024])
            ident_sb = cpool.tile([128, 128], f16, tag="ident")
            ones_sb = cpool.tile([128, 128], f16, tag="ones")
            maskD_sb = cpool.tile([128, 128], f16, tag="maskD")
            wdeltas = sorted(wmask_names)
            wmask_sb = {
                delta: cpool.tile([128, 128], f16, tag=f"maskW{delta}", name=f"mW{delta}")
                for delta in wdeltas
            }
            wpair_sb = None
            if len(wdeltas) == 2 and wdeltas[1] == wdeltas[0] + 1:
                wpair_sb = cpool.tile([128, 256], f16, tag="maskWpair")

            def load_consts_rest():
                nc.sync.dma_start(out=ident_sb[:], in_=ident[:])
                nc.sync.dma_start(out=ones_sb[:], in_=ones[:])
                nc.sync.dma_start(out=maskD_sb[:], in_=maskD[:])
                for delta, m in wmask_sb.items():
                    nc.sync.dma_start(out=m[:], in_=wmask_names[delta][:])
                if wpair_sb is not None:
                    nc.sync.dma_start(out=wpair_sb[:, 0:128], in_=wmask_names[wdeltas[0]][:])
                    nc.sync.dma_start(out=wpair_sb[:, 128:256], in_=wmask_names[wdeltas[1]][:])
                for c2 in range(1, s // 1024):
                    sl2 = slice(c2 * 1024, (c2 + 1) * 1024)
                    nc.sync.dma_start(out=cexp_sb[:, sl2], in_=cexpT[:, sl2])
                    nc.sync.dma_start(out=ssig_sb[:, sl2], in_=ssigT[:, sl2])

            def load(u, first=False):
                qraw = bigpool.tile([128, s], f16, tag="qraw")
                qsw = bigpool.tile([128, s], f16, tag="qsw")
                kraw = bigpool.tile([128, s], f16, tag="kraw")
                ksw = bigpool.tile([128, s], f16, tag="ksw")
                v_sb = bigpool.tile([128, s], f16, tag="v")
                for c2 in range(s // 1024):
                    sl2 = slice(c2 * 1024, (c2 + 1) * 1024)
                    nc.sync.dma_start(out=qraw[:, sl2], in_=qT[u][:, sl2])
                    nc.sync.dma_start(out=qsw[:, sl2], in_=qTs[u][:, sl2])
                    nc.sync.dma_start(out=kraw[:, sl2], in_=kT[u][:, sl2])
                    nc.sync.dma_start(out=ksw[:, sl2], in_=kTs[u][:, sl2])
                nc.sync.dma_start(
                    out=v_sb[:].rearrange("p (n d) -> p n d", d=128),
                    in_=v[u].rearrange("(n p) d -> p n d", p=128),
                )
                rq = bigpool.tile([128, s], f16, tag="rq")
                rk = bigpool.tile([128, s], f16, tag="rk")
                return qraw, qsw, kraw, ksw, v_sb, rq, rk

            def rotary(tiles, lo, hi):
                """Rotary for columns [lo,hi): r = raw*cexp + swapped*ssig.
                raw is reused as scratch for the second product."""
                qraw, qsw, kraw, ksw, v_sb, rq, rk = tiles
                step = 1024 if (hi - lo) % 1024 == 0 else 512
                for raw, sw, r in ((qraw, qsw, rq), (kraw, ksw, rk)):
                    for c in range(lo // step, hi // step):
                        sl = slice(c * step, (c + 1) * step)
                        nc.vector.tensor_mul(r[:, sl], raw[:, sl], cexp_sb[:, sl])
                        nc.vector.tensor_mul(raw[:, sl], sw[:, sl], ssig_sb[:, sl])
                        nc.vector.tensor_add(r[:, sl], r[:, sl], raw[:, sl])

            def emit_masks(probs, kinds, off):
                mk = [k for k in kinds if k[1] != "full"]
                j = 0
                while j < len(mk):
                    t, kind = mk[j]
                    if (
                        wpair_sb is not None
                        and j + 1 < len(mk)
                        and kind != "diag"
                        and mk[j + 1][1] != "diag"
                        and mk[j + 1][0] == t + 1
                        and kind[1] == wdeltas[0]
                    ):
                        tsl = slice(off + t * 128, off + (t + 2) * 128)
                        nc.vector.tensor_mul(probs[:, tsl], probs[:, tsl], wpair_sb[:])
                        j += 2
                        continue
                    m = maskD_sb if kind == "diag" else wmask_sb[kind[1]]
                    tsl = slice(off + t * 128, off + (t + 1) * 128)
                    nc.vector.tensor_mul(probs[:, tsl], probs[:, tsl], m[:])
                    j += 1

            def qb_order(qb):
                kjs = kj_list(qb, s=s, w=w)
                tiles = []
                for kj in kjs:
                    kinds = chunk_kinds(qb, kj, w=w)
                    assert kinds, (qb, kj)
                    tiles.append((kj, kinds, kinds[0][0], kinds[-1][0] + 1))
                fulls = [x for x in tiles if x[3] - x[2] == NQC]
                parts = [x for x in tiles if x[3] - x[2] != NQC]
                assert fulls[0][0] == 0
                order = [fulls[0]]
                rest_f = fulls[1:]
                rest_p = list(parts)
                stride = (
                    max(1, len(rest_f) // (len(rest_p) + 1)) if rest_p else len(rest_f) or 1
                )
                while rest_f or rest_p:
                    order.extend(rest_f[:stride])
                    rest_f = rest_f[stride:]
                    if rest_p:
                        order.append(rest_p.pop(0))
                return order

            WAVE = 4
            state = {"pv": [], "tail": None, "exp_ctr": 0}

            def flush_pv():
                if state["pv"]:
                    state["pv"].pop(0)()

            def flush_all():
                while state["pv"]:
                    flush_pv()

            def attention_qb(u, rq, rk, v_sb, qb):
                order = qb_order(qb)
                n_tiles = len(order)
                qbctx = {}

                def get_psums():
                    if "outT" not in qbctx:
                        outT_ps = ps_out.tile([128, QB], dt.float32, tag="outT")
                        den_ps = ps_den.tile([128, QB], dt.float32, tag="den")
                        qbctx["outT"] = outT_ps
                        qbctx["den"] = den_ps
                    return qbctx["outT"], qbctx["den"]

                for w0 in range(0, n_tiles, WAVE):
                    wave = order[w0 : w0 + WAVE]
                    wprobs = []
                    for kj, kinds, t0, t1 in wave:
                        csl = slice(qb * QB + t0 * 128, qb * QB + t1 * 128)
                        psl = slice(t0 * 128, t1 * 128)
                        ksl = slice(kj * 128, (kj + 1) * 128)
                        sc = ps_sc.tile([128, QB], dt.float32, tag="sc")
                        nc.tensor.matmul(
                            sc[:, psl], rk[:, ksl], rq[:, csl], start=True, stop=True
                        )
                        probs = ppool.tile([128, QB], f16, tag="probs")
                        state["exp_ctr"] += 1
                        if state["exp_ctr"] % DVE_EXP_EVERY == 0:
                            nc.vector.tensor_scalar(
                                probs[:, psl].bitcast(dt.int16),
                                sc[:, psl],
                                SCHRA_A,
                                SCHRA_B,
                                op0=mybir.AluOpType.mult,
                                op1=mybir.AluOpType.add,
                            )
                        else:
                            nc.scalar.activation(
                                probs[:, psl],
                                sc[:, psl],
                                mybir.ActivationFunctionType.Exp,
                                scale=SCALE,
                            )
                        emit_masks(probs, kinds, 0)
                        wprobs.append(probs)

                    is_last_wave = w0 + WAVE >= n_tiles

                    def pv_emit(
                        u=u, qb=qb, wave=wave, wprobs=wprobs,
                        w0=w0, n_tiles=n_tiles, last_wave=is_last_wave,
                    ):
                        outT_ps, den_ps = get_psums()
                        for wi, (kj, kinds, t0, t1) in enumerate(wave):
                            psl = slice(t0 * 128, t1 * 128)
                            ksl = slice(kj * 128, (kj + 1) * 128)
                            first = kj == 0
                            last = w0 + wi == n_tiles - 1
                            nc.tensor.matmul(
                                outT_ps[:, psl], v_sb[:, ksl], wprobs[wi][:, psl],
                                start=first, stop=last,
                            )
                            nc.tensor.matmul(
                                den_ps[:, psl], ones_sb[:], wprobs[wi][:, psl],
                                start=first, stop=last,
                            )
                        if last_wave:
                            # normalize now; transposes/store deferred one qb
                            rden = tpool.tile([128, QB], dt.float32, tag="rden")
                            nc.vector.reciprocal_approx_fast(rden[:], den_ps[:])
                            outN = tpool.tile([128, QB], f16, tag="outN")
                            nc.vector.tensor_mul(outN[:], outT_ps[:], rden[:])

                            def tail(u=u, qb=qb, outN=outN, chunked=False):
                                tr = ps_sc.tile([128, QB], f16, tag="sc")
                                out_sb = tpool.tile([128, QB], f16, tag="out_sb")
                                out_v = out[u].rearrange("(n p) d -> p n d", p=128)
                                if chunked:
                                    for c in range(NQC):
                                        tsl = slice(c * 128, (c + 1) * 128)
                                        nc.tensor.transpose(
                                            tr[:, tsl], outN[:, tsl], ident_sb[:]
                                        )
                                        nc.vector.tensor_copy(
                                            out_sb[:, tsl], tr[:, tsl]
                                        )
                                        nc.sync.dma_start(
                                            out=out_v[:, qb * NQC + c, :],
                                            in_=out_sb[:, tsl],
                                        )
                                    return
                                for c in range(NQC):
                                    tsl = slice(c * 128, (c + 1) * 128)
                                    nc.tensor.transpose(
                                        tr[:, tsl], outN[:, tsl], ident_sb[:]
                                    )
                                nc.vector.tensor_copy(out_sb[:], tr[:])
                                nc.sync.dma_start(
                                    out=out_v[:, qb * NQC : (qb + 1) * NQC, :],
                                    in_=out_sb[:].rearrange("p (n d) -> p n d", d=128),
                                )

                            if state["tail"] is not None:
                                state["tail"]()
                            state["tail"] = tail

                    state["pv"].append(pv_emit)
                    flush_pv() if len(state["pv"]) > 1 else None

            cur = load(0)
            load_consts_rest()
            # warm the ACT exp table during the load phase so the first real
            # exp doesn't pay the table-load latency
            warm = tpool.tile([128, 1], f16, tag="warm")
            nc.scalar.activation(
                warm[:], ones_sb[:, 0:1], mybir.ActivationFunctionType.Exp, scale=1.0
            )
            for u in range(per_core):
                nxt = load(u + 1) if u + 1 < per_core else None
                if u == 0:
                    rotary(cur, 0, QB)
                for qb in range(nqb):
                    if u == 0 and qb + 1 < nqb:
                        rotary(cur, (qb + 1) * QB, (qb + 2) * QB)
                    attention_qb(u, cur[5], cur[6], cur[4], qb)
                if nxt is not None:
                    rotary(nxt, 0, s)
                cur = nxt
            flush_all()
            if state["tail"] is not None:
                state["tail"](chunked=True)

    nc.compile()
    return nc


def host_prep(q, k, v, cos, sin, s=S, w=LOCAL_WINDOW):
    """Build per-core input maps from full inputs (fp16 device layouts)."""
    b, _, h, d = q.shape
    partial = _window_partial_deltas(w)
    f16 = np.float16

    cexp = np.empty((128, s), dtype=np.float32)
    ssig = np.empty((128, s), dtype=np.float32)
    cexp[0::2, :] = cos.T
    cexp[1::2, :] = cos.T
    ssig[0::2, :] = -sin.T
    ssig[1::2, :] = sin.T

    ident = np.eye(128, dtype=np.float32)
    ones = np.ones((128, 128), dtype=np.float32)

    p = np.arange(128)[:, None]
    c = np.arange(128)[None, :]
    maskD = (c >= p).astype(np.float32)
    wmasks = {delta: ((c - p) < t).astype(np.float32) for delta, t in partial.items()}

    perm = np.arange(128) ^ 1  # rotary pair swap on the d axis

    units = [(bi, hi) for bi in range(b) for hi in range(h)]
    per = len(units) // N_CORES
    in_maps = []
    for core in range(N_CORES):
        us = units[core * per : (core + 1) * per]
        qTc = np.stack([q[bi, :, hi, :].T for bi, hi in us]).astype(f16)
        kTc = np.stack([k[bi, :, hi, :].T for bi, hi in us]).astype(f16)
        vc = np.stack([v[bi, :, hi, :] for bi, hi in us]).astype(f16)
        m = {
            "qT": np.ascontiguousarray(qTc),
            "qTs": np.ascontiguousarray(qTc[:, perm, :]),
            "kT": np.ascontiguousarray(kTc),
            "kTs": np.ascontiguousarray(kTc[:, perm, :]),
            "v": np.ascontiguousarray(vc),
            "cexpT": cexp.astype(f16),
            "ssigT": ssig.astype(f16),
            "ident": ident.astype(f16),
            "ones": ones.astype(f16),
            "maskD": maskD.astype(f16),
        }
        for delta, msk in wmasks.items():
            m[f"maskW{delta}"] = msk.astype(f16)
        in_maps.append(m)
    return in_maps, units


_NC_CACHE = {}


def kernel(q, k, v, cos, sin):
    from concourse.bass_utils import run_bass_kernel_spmd

    q = np.asarray(q, dtype=np.float32)
    k = np.asarray(k, dtype=np.float32)
    v = np.asarray(v, dtype=np.float32)
    cos = np.asarray(cos, dtype=np.float32)
    sin = np.asarray(sin, dtype=np.float32)

    if "nc" not in _NC_CACHE:
        _NC_CACHE["nc"] = build_nc()
    nc = _NC_CACHE["nc"]

    in_maps, units = host_prep(q, k, v, cos, sin)
    res = run_bass_kernel_spmd(nc, in_maps, core_ids=list(range(N_CORES)))

    b, s, h, d = q.shape
    full = np.empty((b, s, h, d), dtype=np.float32)
    per = len(units) // N_CORES
    for core in range(N_CORES):
        o = np.asarray(res.results[core]["out"], dtype=np.float32)  # [per, s, 128]
        for i, (bi, hi) in enumerate(units[core * per : (core + 1) * per]):
            full[bi, :, hi, :] = o[i]
    return full
